# revision 12
# baseline (speedup 1.0000x reference)
"""MoE FFN (SwiGLU, E=8 experts, top-2) + shared expert, expert-parallel
across 8 NeuronCores, with host-side token routing.

Strategy: the gate (softmax -> top-2 -> renormalize) is computed on host
with the exact same jax ops as the reference (on CPU), which yields per
expert the list of selected tokens and their combine weights.  Core e
owns expert e and computes the SwiGLU FFN only over the ~T*2/8 tokens
routed to it (gathered+padded to capacity C, a multiple of 128), scaled
by the combine weight on device.  Each core additionally computes a
344-row shard of the shared expert's hidden dim over all T tokens.
Host gathers: out = sum_cores shared_partial; out[idx_e] += expert_out_e.

All FFN matmuls run in bf16 with fp32 PSUM accumulation.  Expert hidden
is tiled 21x128+64 (no padding compute), shared shard 128+128+88.
Weight DMAs ride the Pool-engine queue, x tiles the SP queue, output
tiles the ACT queue, so they overlap.
"""
import numpy as np
from contextlib import ExitStack

D, E, T = 1024, 8, 4096
NK = 8                              # D / 128 contraction tiles
TN = 512                            # token tile (moving free axis)
NTN = T // TN                       # 8 shared-expert token tiles
H = 2752
HM_W = [128] * 21 + [64]            # expert hidden tile widths (sum 2752)
NHM = len(HM_W)
HS = 344                            # shared-expert hidden shard per core
HS_W = [128, 128, 88]               # shard tile widths (sum 344)
NHS = len(HS_W)
HSP = 384                           # padded shard rows in DRAM layout

_CACHE = {}


def _route(xt, gate_w):
    """Top-2 routing, mirroring reference ops bit-for-bit on CPU jax."""
    try:
        import jax
        import jax.numpy as jnp
        cpu = jax.devices("cpu")[0]
        with jax.default_device(cpu):
            logits = jnp.asarray(xt) @ jnp.asarray(gate_w).T
            scores = jax.nn.softmax(logits, axis=-1)
            tw, ti = jax.lax.top_k(scores, 2)
            tw = tw / (jnp.sum(tw, axis=-1, keepdims=True) + 1e-20)
        return np.asarray(tw), np.asarray(ti)
    except Exception:
        lg = xt.astype(np.float64) @ gate_w.astype(np.float64).T
        sc = np.exp(lg - lg.max(-1, keepdims=True))
        sc /= sc.sum(-1, keepdims=True)
        ti = np.argsort(-sc, axis=-1, kind="stable")[:, :2]
        tw = np.take_along_axis(sc, ti, axis=-1)
        tw = tw / (tw.sum(-1, keepdims=True) + 1e-20)
        return tw.astype(np.float32), ti.astype(np.int32)


def _build_nc(reps=1, C=None, sched="v3"):
    import concourse.bass as bass
    import concourse.tile as tile
    from concourse import bacc, mybir

    if C is None:
        C = _CACHE.get("C", 1152)
    NET = (C + TN - 1) // TN            # expert token tiles (DRAM padded to 512)
    CP = NET * TN
    ET_W = [min(TN, C - t * TN) for t in range(NET)]   # compute widths

    f32 = mybir.dt.float32
    bf16 = mybir.dt.bfloat16
    ALU = mybir.AluOpType
    ACT = mybir.ActivationFunctionType

    nc = bacc.Bacc("TRN2", target_bir_lowering=False, debug=False, num_devices=8)

    xsr = nc.dram_tensor("xs", [128, NTN, NK, TN], bf16, kind="ExternalInput").ap()
    xer = nc.dram_tensor("xe", [128, NET, NK, TN], bf16, kind="ExternalInput").ap()
    cwr = nc.dram_tensor("cw", [128, CP // 128], f32, kind="ExternalInput").ap()
    w1r = nc.dram_tensor("w1", [128, NHM, NK, 128], bf16, kind="ExternalInput").ap()
    w3r = nc.dram_tensor("w3", [128, NHM, NK, 128], bf16, kind="ExternalInput").ap()
    w2r = nc.dram_tensor("w2", [128, NHM, 1024], bf16, kind="ExternalInput").ap()
    s1r = nc.dram_tensor("s1", [128, NHS, NK, 128], bf16, kind="ExternalInput").ap()
    s3r = nc.dram_tensor("s3", [128, NHS, NK, 128], bf16, kind="ExternalInput").ap()
    s2r = nc.dram_tensor("s2", [128, NHS, 1024], bf16, kind="ExternalInput").ap()
    oer = nc.dram_tensor("oe", [CP, D], bf16, kind="ExternalOutput").ap()
    osr = nc.dram_tensor("os", [T, D], bf16, kind="ExternalOutput").ap()

    with tile.TileContext(nc) as tc, ExitStack() as ctx:
        wpool = ctx.enter_context(tc.tile_pool(name="w", bufs=1))
        w1sb = wpool.tile([128, NHM, NK, 128], bf16, tag="w1")
        w3sb = wpool.tile([128, NHM, NK, 128], bf16, tag="w3")
        w2sb = wpool.tile([128, NHM, 1024], bf16, tag="w2")
        s1sb = wpool.tile([128, NHS, NK, 128], bf16, tag="s1")
        s3sb = wpool.tile([128, NHS, NK, 128], bf16, tag="s3")
        s2sb = wpool.tile([128, NHS, 1024], bf16, tag="s2")
        cwsb = wpool.tile([128, CP // 128], f32, tag="cw")

        # shared-expert weights first (needed by the first work items);
        # big expert weights spread across three DGE queues (Pool, DVE,
        # ACT) so they land in ~1/3 the time for single-shot latency
        nc.gpsimd.dma_start(s1sb[:], s1r[:])
        nc.gpsimd.dma_start(s3sb[:], s3r[:])
        nc.scalar.dma_start(s2sb[:], s2r[:])
        nc.gpsimd.dma_start(w1sb[:], w1r[:])
        nc.scalar.dma_start(w3sb[:], w3r[:])
        nc.gpsimd.dma_start(w2sb[:], w2r[:])
        nc.sync.dma_start(cwsb[:], cwr[:])

        xp = ctx.enter_context(tc.tile_pool(name="x", bufs=2))
        hp = ctx.enter_context(tc.tile_pool(name="h", bufs=1))
        shp = ctx.enter_context(tc.tile_pool(name="sh", bufs=2))
        silp = ctx.enter_context(tc.tile_pool(name="sil", bufs=2))
        obp = ctx.enter_context(tc.tile_pool(name="ob", bufs=3))
        pfw = ctx.enter_context(tc.tile_pool(name="pfw", bufs=1, space="PSUM"))
        pyo = ctx.enter_context(tc.tile_pool(name="pyo", bufs=2, space="PSUM"))
        # pfw: tags g0,g1,g2 x1 buf = 3 banks; pyo: tags y0,y1 x2 bufs = 4 banks

        def emit_item(shared, idx):
            if shared:
                hw_list, wa, wb, wc = HS_W, s1sb, s3sb, s2sb
                xsrc, w = xsr[:, idx], TN
            else:
                hw_list, wa, wb, wc = HM_W, w1sb, w3sb, w2sb
                xsrc, w = xer[:, idx], ET_W[idx]
            nh = len(hw_list)

            xt = xp.tile([128, NK, TN], bf16, tag="x")
            nc.sync.dma_start(xt[:], xsrc)

            if shared:
                hbuf = shp.tile([128, NHS, TN], bf16, tag="shT")
            else:
                hbuf = hp.tile([128, NHM, TN], bf16, tag="hT")

            if sched == "v2":
                for hm in range(nh):
                    wh = hw_list[hm]
                    p1 = pfw.tile([128, TN], f32, tag="g0", name="p1")
                    p3 = pfw.tile([128, TN], f32, tag="g1", name="p3")
                    for kk in range(NK):
                        nc.tensor.matmul(p1[:wh, :w], wa[:, hm, kk, :wh],
                                         xt[:, kk, :w],
                                         start=(kk == 0), stop=(kk == NK - 1))
                        nc.tensor.matmul(p3[:wh, :w], wb[:, hm, kk, :wh],
                                         xt[:, kk, :w],
                                         start=(kk == 0), stop=(kk == NK - 1))
                    sil = silp.tile([128, TN], f32, tag="sil", name="sil")
                    nc.scalar.activation(sil[:wh, :w], p1[:wh, :w], ACT.Silu)
                    nc.vector.tensor_mul(hbuf[:wh, hm, :w], sil[:wh, :w],
                                         p3[:wh, :w])
                for tm in range(w // 128):
                    ob = obp.tile([128, D], bf16, tag="ob", name="ob")
                    y0 = pyo.tile([128, 512], f32, tag="y0", name="y0")
                    y1 = pyo.tile([128, 512], f32, tag="y1", name="y1")
                    for hk in range(nh):
                        wh = hw_list[hk]
                        st = hbuf[:wh, hk, tm * 128:(tm + 1) * 128]
                        nc.tensor.matmul(y0[:], st, wc[:wh, hk, 0:512],
                                         start=(hk == 0), stop=(hk == nh - 1))
                        nc.tensor.matmul(y1[:], st, wc[:wh, hk, 512:1024],
                                         start=(hk == 0), stop=(hk == nh - 1))
                    for dn, yo in ((0, y0), (1, y1)):
                        osl = ob[:, dn * 512:(dn + 1) * 512]
                        if shared:
                            nc.scalar.activation(osl, yo[:], ACT.Copy)
                        else:
                            cb = idx * (TN // 128) + tm
                            nc.vector.tensor_scalar(osl, yo[:],
                                                    cwsb[:, cb:cb + 1],
                                                    None, op0=ALU.mult)
                    t0 = idx * TN + tm * 128
                    dst = osr[t0:t0 + 128, :] if shared else oer[t0:t0 + 128, :]
                    nc.scalar.dma_start(dst, ob[:])
                return

            # h1/h3 PSUM chains run as rolling groups of 3 interleaved
            # chains: consecutive matmuls hit different PSUM banks with
            # different stationaries, dodging the same-bank accumulation
            # stall (427ns/MM sequential -> ~239ns/MM at 3-way).
            chains = []
            for hm in range(nh):
                chains.append((hm, 0))
                chains.append((hm, 1))
            sil_t = {}
            for c0 in range(0, len(chains), 3):
                grp = chains[c0:c0 + 3]
                tiles = []
                for s in range(len(grp)):
                    gt = f"g{(c0 + s) % 4}"   # 4 rotating bank slots
                    pt = pfw.tile([128, TN], f32, tag=gt, name=f"pt{s}")
                    tiles.append(pt)
                for kk in range(NK):
                    for s, (hm, kind) in enumerate(grp):
                        wh = hw_list[hm]
                        wsrc = wa if kind == 0 else wb
                        nc.tensor.matmul(tiles[s][:wh, :w],
                                         wsrc[:, hm, kk, :wh], xt[:, kk, :w],
                                         start=(kk == 0), stop=(kk == NK - 1))
                for s, (hm, kind) in enumerate(grp):
                    wh = hw_list[hm]
                    if kind == 0:
                        sil = silp.tile([128, TN], f32, tag="sil", name="sil")
                        nc.scalar.activation(sil[:wh, :w], tiles[s][:wh, :w],
                                             ACT.Silu)
                        sil_t[hm] = sil
                    else:
                        nc.vector.tensor_mul(hbuf[:wh, hm, :w],
                                             sil_t.pop(hm)[:wh, :w],
                                             tiles[s][:wh, :w])

            for tm in range(w // 128):
                ob = obp.tile([128, D], bf16, tag="ob")
                # dn=0 / dn=1 chains interleaved with a one-step offset so
                # consecutive matmuls differ in both bank and stationary.
                y0 = pyo.tile([128, 512], f32, tag="y0")
                y1 = pyo.tile([128, 512], f32, tag="y1")

                def stat(hk):
                    return hbuf[:hw_list[hk], hk, tm * 128:(tm + 1) * 128]

                nc.tensor.matmul(y0[:], stat(0), wc[:hw_list[0], 0, 0:512],
                                 start=True, stop=(nh == 1))
                for hk in range(1, nh):
                    nc.tensor.matmul(y0[:], stat(hk),
                                     wc[:hw_list[hk], hk, 0:512],
                                     start=False, stop=(hk == nh - 1))
                    nc.tensor.matmul(y1[:], stat(hk - 1),
                                     wc[:hw_list[hk - 1], hk - 1, 512:1024],
                                     start=(hk == 1), stop=False)
                nc.tensor.matmul(y1[:], stat(nh - 1),
                                 wc[:hw_list[nh - 1], nh - 1, 512:1024],
                                 start=(nh == 1), stop=True)
                for dn, yo in ((0, y0), (1, y1)):
                    osl = ob[:, dn * 512:(dn + 1) * 512]
                    if shared:
                        nc.scalar.activation(osl, yo[:], ACT.Copy)
                    else:
                        cb = idx * (TN // 128) + tm
                        nc.vector.tensor_scalar(osl, yo[:], cwsb[:, cb:cb + 1],
                                                None, op0=ALU.mult)
                t0 = idx * TN + tm * 128
                dst = osr[t0:t0 + 128, :] if shared else oer[t0:t0 + 128, :]
                nc.scalar.dma_start(dst, ob[:])

        # interleave: 2 shared items first (their weights+x land first),
        # then alternate expert/shared so dependency bubbles overlap
        order = [("s", 0), ("s", 1)]
        si, ei = 2, 0
        while ei < NET or si < NTN:
            if ei < NET:
                order.append(("e", ei)); ei += 1
            if si < NTN:
                order.append(("s", si)); si += 1

        for _rep in range(reps):
            for kind, idx in order:
                emit_item(kind == "s", idx)

    nc.compile()
    return nc


def _prep_inputs(x, gate_w, w1, w3, w2, sw1, sw3, sw2):
    import ml_dtypes
    bf16 = ml_dtypes.bfloat16

    xf = np.ascontiguousarray(x.reshape(T, D).astype(np.float32))
    topk_w, topk_idx = _route(xf, np.asarray(gate_w, np.float32))

    sels, wsels = [], []
    for e in range(E):
        m = topk_idx == e                    # [T, 2]
        sel = np.nonzero(m.any(1))[0]
        wv = np.where(m[:, 0], topk_w[:, 0], topk_w[:, 1])[sel]
        sels.append(sel)
        wsels.append(wv.astype(np.float32))
    cnts = [len(s) for s in sels]
    C = max(512, -(-max(cnts) // 128) * 128)
    NET = (C + TN - 1) // TN
    CP = NET * TN

    _CACHE["C"] = C
    _CACHE["route"] = (sels, cnts)

    # [128p, NTN, NK, TN]: X[p,tn,kk,c] = xf[tn*TN+c, kk*128+p]
    xsr = np.ascontiguousarray(
        xf.reshape(NTN, TN, NK, 128).transpose(3, 0, 2, 1).astype(bf16))

    def pad_rows(a, n):
        return np.concatenate(
            [a, np.zeros((n - a.shape[0],) + a.shape[1:], a.dtype)], 0)

    in_maps = []
    for e in range(E):
        xg = pad_rows(xf[sels[e]], CP)                    # [CP, D]
        xer = np.ascontiguousarray(
            xg.reshape(NET, TN, NK, 128).transpose(3, 0, 2, 1).astype(bf16))
        cwv = pad_rows(wsels[e], CP)                      # [CP]
        cwr = np.ascontiguousarray(cwv.reshape(CP // 128, 128).T)

        w1p = pad_rows(np.asarray(w1[e], np.float32), NHM * 128)   # [2816, D]
        w3p = pad_rows(np.asarray(w3[e], np.float32), NHM * 128)
        w2p = np.asarray(w2[e], np.float32)                        # [D, H]
        w2p = np.concatenate(
            [w2p, np.zeros((D, NHM * 128 - H), np.float32)], 1)

        w1r = np.ascontiguousarray(
            w1p.T.reshape(NK, 128, NHM, 128).transpose(1, 2, 0, 3).astype(bf16))
        w3r = np.ascontiguousarray(
            w3p.T.reshape(NK, 128, NHM, 128).transpose(1, 2, 0, 3).astype(bf16))
        w2r = np.ascontiguousarray(
            w2p.T.reshape(NHM, 128, D).transpose(1, 0, 2).astype(bf16))

        s1p = pad_rows(np.asarray(sw1[e * HS:(e + 1) * HS], np.float32), HSP)
        s3p = pad_rows(np.asarray(sw3[e * HS:(e + 1) * HS], np.float32), HSP)
        s2p = np.asarray(sw2[:, e * HS:(e + 1) * HS], np.float32)
        s2p = np.concatenate([s2p, np.zeros((D, HSP - HS), np.float32)], 1)

        s1rr = np.ascontiguousarray(
            s1p.T.reshape(NK, 128, NHS, 128).transpose(1, 2, 0, 3).astype(bf16))
        s3rr = np.ascontiguousarray(
            s3p.T.reshape(NK, 128, NHS, 128).transpose(1, 2, 0, 3).astype(bf16))
        s2rr = np.ascontiguousarray(
            s2p.T.reshape(NHS, 128, D).transpose(1, 0, 2).astype(bf16))

        in_maps.append({
            "xs": xsr, "xe": xer, "cw": cwr,
            "w1": w1r, "w3": w3r, "w2": w2r,
            "s1": s1rr, "s3": s3rr, "s2": s2rr,
        })
    return in_maps


def _run(in_maps, trace=False):
    from concourse.bass_utils import run_bass_kernel_spmd
    C = _CACHE["C"]
    if _CACHE.get("nc_C") != C:
        _CACHE["nc"] = _build_nc(C=C)
        _CACHE["nc_C"] = C
    nc = _CACHE["nc"]
    return run_bass_kernel_spmd(nc, in_maps, list(range(E)), trace=trace)


def kernel(x, gate_w, w1, w3, w2, sw1, sw3, sw2):
    in_maps = _prep_inputs(x, gate_w, w1, w3, w2, sw1, sw3, sw2)
    res = _run(in_maps)
    sels, cnts = _CACHE["route"]
    total = np.zeros((T, D), np.float32)
    for r in res.results:
        total += r["os"].astype(np.float32)
    for e, r in enumerate(res.results):
        total[sels[e]] += r["oe"][:cnts[e]].astype(np.float32)
    return total.reshape(x.shape).astype(x.dtype)


# revision 16
# speedup vs baseline: 1.1028x; 1.1028x over previous
"""MoE FFN (SwiGLU, E=8 experts, top-2) + shared expert, expert-parallel
across 8 NeuronCores, with host-side token routing.

Strategy: the gate (softmax -> top-2 -> renormalize) is computed on host
with the exact same jax ops as the reference (on CPU), which yields per
expert the list of selected tokens and their combine weights.  Core e
owns expert e and computes the SwiGLU FFN only over the ~T*2/8 tokens
routed to it (gathered+padded to capacity C, a multiple of 128), scaled
by the combine weight on device.  Each core additionally computes a
344-row shard of the shared expert's hidden dim over all T tokens.
Host gathers: out = sum_cores shared_partial; out[idx_e] += expert_out_e.

All FFN matmuls run in bf16 with fp32 PSUM accumulation.  Expert hidden
is tiled 21x128+64 (no padding compute), shared shard 128+128+88.
Weight DMAs ride the Pool-engine queue, x tiles the SP queue, output
tiles the ACT queue, so they overlap.
"""
import numpy as np
from contextlib import ExitStack

D, E, T = 1024, 8, 4096
NK = 8                              # D / 128 contraction tiles
TN = 512                            # token tile (moving free axis)
NTN = T // TN                       # 8 shared-expert token tiles
H = 2752
HM_W = [128] * 21 + [64]            # expert hidden tile widths (sum 2752)
NHM = len(HM_W)
HS = 344                            # shared-expert hidden shard per core
HS_W = [128, 128, 88]               # shard tile widths (sum 344)
NHS = len(HS_W)
HSP = 384                           # padded shard rows in DRAM layout

_CACHE = {}


def _route(xt, gate_w):
    """Top-2 routing, mirroring reference ops bit-for-bit on CPU jax."""
    try:
        import jax
        import jax.numpy as jnp
        cpu = jax.devices("cpu")[0]
        with jax.default_device(cpu):
            logits = jnp.asarray(xt) @ jnp.asarray(gate_w).T
            scores = jax.nn.softmax(logits, axis=-1)
            tw, ti = jax.lax.top_k(scores, 2)
            tw = tw / (jnp.sum(tw, axis=-1, keepdims=True) + 1e-20)
        return np.asarray(tw), np.asarray(ti)
    except Exception:
        lg = xt.astype(np.float64) @ gate_w.astype(np.float64).T
        sc = np.exp(lg - lg.max(-1, keepdims=True))
        sc /= sc.sum(-1, keepdims=True)
        ti = np.argsort(-sc, axis=-1, kind="stable")[:, :2]
        tw = np.take_along_axis(sc, ti, axis=-1)
        tw = tw / (tw.sum(-1, keepdims=True) + 1e-20)
        return tw.astype(np.float32), ti.astype(np.int32)


def _build_nc(reps=1, C=None):
    import concourse.bass as bass
    import concourse.tile as tile
    from concourse import bacc, mybir

    if C is None:
        C = _CACHE.get("C", 1152)
    NET = (C + TN - 1) // TN            # expert token tiles (DRAM padded to 512)
    CP = NET * TN
    ET_W = [min(TN, C - t * TN) for t in range(NET)]   # compute widths

    f32 = mybir.dt.float32
    bf16 = mybir.dt.bfloat16
    ALU = mybir.AluOpType
    ACT = mybir.ActivationFunctionType

    nc = bacc.Bacc("TRN2", target_bir_lowering=False, debug=False, num_devices=8)

    xsr = nc.dram_tensor("xs", [128, NTN, NK, TN], bf16, kind="ExternalInput").ap()
    xer = nc.dram_tensor("xe", [128, NET, NK, TN], bf16, kind="ExternalInput").ap()
    cwr = nc.dram_tensor("cw", [128, CP // 128], f32, kind="ExternalInput").ap()
    w1r = nc.dram_tensor("w1", [128, NHM, NK, 128], bf16, kind="ExternalInput").ap()
    w3r = nc.dram_tensor("w3", [128, NHM, NK, 128], bf16, kind="ExternalInput").ap()
    w2r = nc.dram_tensor("w2", [128, NHM, 1024], bf16, kind="ExternalInput").ap()
    s1r = nc.dram_tensor("s1", [128, NHS, NK, 128], bf16, kind="ExternalInput").ap()
    s3r = nc.dram_tensor("s3", [128, NHS, NK, 128], bf16, kind="ExternalInput").ap()
    s2r = nc.dram_tensor("s2", [128, NHS, 1024], bf16, kind="ExternalInput").ap()
    oer = nc.dram_tensor("oe", [CP, D], bf16, kind="ExternalOutput").ap()
    osr = nc.dram_tensor("os", [T, D], bf16, kind="ExternalOutput").ap()

    with tile.TileContext(nc) as tc, ExitStack() as ctx:
        wpool = ctx.enter_context(tc.tile_pool(name="w", bufs=1))
        w1sb = wpool.tile([128, NHM, NK, 128], bf16, tag="w1")
        w3sb = wpool.tile([128, NHM, NK, 128], bf16, tag="w3")
        w2sb = wpool.tile([128, NHM, 1024], bf16, tag="w2")
        s1sb = wpool.tile([128, NHS, NK, 128], bf16, tag="s1")
        s3sb = wpool.tile([128, NHS, NK, 128], bf16, tag="s3")
        s2sb = wpool.tile([128, NHS, 1024], bf16, tag="s2")
        cwsb = wpool.tile([128, CP // 128], f32, tag="cw")

        # shared-expert weights first (needed by the first work items);
        # big expert weights spread across three DGE queues (Pool, DVE,
        # ACT) so they land in ~1/3 the time for single-shot latency
        nc.gpsimd.dma_start(s1sb[:], s1r[:])
        nc.gpsimd.dma_start(s3sb[:], s3r[:])
        nc.scalar.dma_start(s2sb[:], s2r[:])
        nc.gpsimd.dma_start(w1sb[:], w1r[:])
        nc.scalar.dma_start(w3sb[:], w3r[:])
        nc.gpsimd.dma_start(w2sb[:], w2r[:])
        nc.sync.dma_start(cwsb[:], cwr[:])

        xp = ctx.enter_context(tc.tile_pool(name="x", bufs=2))
        hp = ctx.enter_context(tc.tile_pool(name="h", bufs=1))
        shp = ctx.enter_context(tc.tile_pool(name="sh", bufs=2))
        silp = ctx.enter_context(tc.tile_pool(name="sil", bufs=2))
        obp = ctx.enter_context(tc.tile_pool(name="ob", bufs=3))
        pfw = ctx.enter_context(tc.tile_pool(name="pfw", bufs=1, space="PSUM"))
        pyo = ctx.enter_context(tc.tile_pool(name="pyo", bufs=2, space="PSUM"))
        # pfw: tags g0,g1,g2 x1 buf = 3 banks; pyo: tags y0,y1 x2 bufs = 4 banks

        def emit_item(shared, idx):
            if shared:
                hw_list, wa, wb, wc = HS_W, s1sb, s3sb, s2sb
                xsrc, w = xsr[:, idx], TN
            else:
                hw_list, wa, wb, wc = HM_W, w1sb, w3sb, w2sb
                xsrc, w = xer[:, idx], ET_W[idx]
            nh = len(hw_list)

            xt = xp.tile([128, NK, TN], bf16, tag="x")
            nc.sync.dma_start(xt[:], xsrc)

            if shared:
                hbuf = shp.tile([128, NHS, TN], bf16, tag="shT")
            else:
                hbuf = hp.tile([128, NHM, TN], bf16, tag="hT")

            # h1/h3 PSUM chains run as rolling groups of 3 interleaved
            # chains: consecutive matmuls hit different PSUM banks with
            # different stationaries, dodging the same-bank accumulation
            # stall (427ns/MM sequential -> ~239ns/MM at 3-way).
            chains = []
            for hm in range(nh):
                chains.append((hm, 0))
                chains.append((hm, 1))
            sil_t = {}
            for c0 in range(0, len(chains), 3):
                grp = chains[c0:c0 + 3]
                tiles = []
                for s in range(len(grp)):
                    gt = f"g{(c0 + s) % 4}"   # 4 rotating bank slots
                    pt = pfw.tile([128, TN], f32, tag=gt, name=f"pt{s}")
                    tiles.append(pt)
                for kk in range(NK):
                    for s, (hm, kind) in enumerate(grp):
                        wh = hw_list[hm]
                        wsrc = wa if kind == 0 else wb
                        nc.tensor.matmul(tiles[s][:wh, :w],
                                         wsrc[:, hm, kk, :wh], xt[:, kk, :w],
                                         start=(kk == 0), stop=(kk == NK - 1))
                for s, (hm, kind) in enumerate(grp):
                    wh = hw_list[hm]
                    if kind == 0:
                        sil = silp.tile([128, TN], f32, tag="sil", name="sil")
                        nc.scalar.activation(sil[:wh, :w], tiles[s][:wh, :w],
                                             ACT.Silu)
                        sil_t[hm] = sil
                    else:
                        nc.vector.tensor_mul(hbuf[:wh, hm, :w],
                                             sil_t.pop(hm)[:wh, :w],
                                             tiles[s][:wh, :w])

            # w2: per tm-block, dn=0 / dn=1 chains interleaved with a
            # one-step offset so consecutive matmuls differ in both bank
            # and stationary.  Each tm's y1 epilogue is deferred into the
            # next tm's stream to avoid back-to-back same-bank matmuls.
            def stat(hk, tm):
                return hbuf[:hw_list[hk], hk, tm * 128:(tm + 1) * 128]

            def emit_consumers(tm, y0, y1, ob):
                for dn, yo in ((0, y0), (1, y1)):
                    osl = ob[:, dn * 512:(dn + 1) * 512]
                    if shared:
                        # DVE, not ACT: ACT already carries the silus plus
                        # output-DMA issue and would lag the PE here
                        nc.vector.tensor_copy(osl, yo[:])
                    else:
                        cb = idx * (TN // 128) + tm
                        nc.vector.tensor_scalar(osl, yo[:], cwsb[:, cb:cb + 1],
                                                None, op0=ALU.mult)
                t0 = idx * TN + tm * 128
                dst = osr[t0:t0 + 128, :] if shared else oer[t0:t0 + 128, :]
                nc.scalar.dma_start(dst, ob[:])

            pending = None   # (tm, y0, y1, ob) awaiting y1 epilogue
            for tm in range(w // 128):
                ob = obp.tile([128, D], bf16, tag="ob")
                y0 = pyo.tile([128, 512], f32, tag="y0")
                y1 = pyo.tile([128, 512], f32, tag="y1")
                nc.tensor.matmul(y0[:], stat(0, tm), wc[:hw_list[0], 0, 0:512],
                                 start=True, stop=(nh == 1))
                if pending is not None:
                    ptm, py0, py1, pob = pending
                    nc.tensor.matmul(py1[:], stat(nh - 1, ptm),
                                     wc[:hw_list[nh - 1], nh - 1, 512:1024],
                                     start=(nh == 1), stop=True)
                    emit_consumers(ptm, py0, py1, pob)
                for hk in range(1, nh):
                    nc.tensor.matmul(y0[:], stat(hk, tm),
                                     wc[:hw_list[hk], hk, 0:512],
                                     start=False, stop=(hk == nh - 1))
                    nc.tensor.matmul(y1[:], stat(hk - 1, tm),
                                     wc[:hw_list[hk - 1], hk - 1, 512:1024],
                                     start=(hk == 1), stop=False)
                pending = (tm, y0, y1, ob)
            ptm, py0, py1, pob = pending
            nc.tensor.matmul(py1[:], stat(nh - 1, ptm),
                             wc[:hw_list[nh - 1], nh - 1, 512:1024],
                             start=(nh == 1), stop=True)
            emit_consumers(ptm, py0, py1, pob)

        # interleave: 2 shared items first (their weights+x land first),
        # then alternate expert/shared so dependency bubbles overlap
        order = [("s", 0), ("s", 1)]
        si, ei = 2, 0
        while ei < NET or si < NTN:
            if ei < NET:
                order.append(("e", ei)); ei += 1
            if si < NTN:
                order.append(("s", si)); si += 1

        for _rep in range(reps):
            for kind, idx in order:
                emit_item(kind == "s", idx)

    nc.compile()
    return nc


def _prep_inputs(x, gate_w, w1, w3, w2, sw1, sw3, sw2):
    import ml_dtypes
    bf16 = ml_dtypes.bfloat16

    xf = np.ascontiguousarray(x.reshape(T, D).astype(np.float32))
    topk_w, topk_idx = _route(xf, np.asarray(gate_w, np.float32))

    sels, wsels = [], []
    for e in range(E):
        m = topk_idx == e                    # [T, 2]
        sel = np.nonzero(m.any(1))[0]
        wv = np.where(m[:, 0], topk_w[:, 0], topk_w[:, 1])[sel]
        sels.append(sel)
        wsels.append(wv.astype(np.float32))
    cnts = [len(s) for s in sels]
    C = max(512, -(-max(cnts) // 128) * 128)
    NET = (C + TN - 1) // TN
    CP = NET * TN

    _CACHE["C"] = C
    _CACHE["route"] = (sels, cnts)

    # [128p, NTN, NK, TN]: X[p,tn,kk,c] = xf[tn*TN+c, kk*128+p]
    xsr = np.ascontiguousarray(
        xf.reshape(NTN, TN, NK, 128).transpose(3, 0, 2, 1).astype(bf16))

    def pad_rows(a, n):
        return np.concatenate(
            [a, np.zeros((n - a.shape[0],) + a.shape[1:], a.dtype)], 0)

    in_maps = []
    for e in range(E):
        xg = pad_rows(xf[sels[e]], CP)                    # [CP, D]
        xer = np.ascontiguousarray(
            xg.reshape(NET, TN, NK, 128).transpose(3, 0, 2, 1).astype(bf16))
        cwv = pad_rows(wsels[e], CP)                      # [CP]
        cwr = np.ascontiguousarray(cwv.reshape(CP // 128, 128).T)

        w1p = pad_rows(np.asarray(w1[e], np.float32), NHM * 128)   # [2816, D]
        w3p = pad_rows(np.asarray(w3[e], np.float32), NHM * 128)
        w2p = np.asarray(w2[e], np.float32)                        # [D, H]
        w2p = np.concatenate(
            [w2p, np.zeros((D, NHM * 128 - H), np.float32)], 1)

        w1r = np.ascontiguousarray(
            w1p.T.reshape(NK, 128, NHM, 128).transpose(1, 2, 0, 3).astype(bf16))
        w3r = np.ascontiguousarray(
            w3p.T.reshape(NK, 128, NHM, 128).transpose(1, 2, 0, 3).astype(bf16))
        w2r = np.ascontiguousarray(
            w2p.T.reshape(NHM, 128, D).transpose(1, 0, 2).astype(bf16))

        s1p = pad_rows(np.asarray(sw1[e * HS:(e + 1) * HS], np.float32), HSP)
        s3p = pad_rows(np.asarray(sw3[e * HS:(e + 1) * HS], np.float32), HSP)
        s2p = np.asarray(sw2[:, e * HS:(e + 1) * HS], np.float32)
        s2p = np.concatenate([s2p, np.zeros((D, HSP - HS), np.float32)], 1)

        s1rr = np.ascontiguousarray(
            s1p.T.reshape(NK, 128, NHS, 128).transpose(1, 2, 0, 3).astype(bf16))
        s3rr = np.ascontiguousarray(
            s3p.T.reshape(NK, 128, NHS, 128).transpose(1, 2, 0, 3).astype(bf16))
        s2rr = np.ascontiguousarray(
            s2p.T.reshape(NHS, 128, D).transpose(1, 0, 2).astype(bf16))

        in_maps.append({
            "xs": xsr, "xe": xer, "cw": cwr,
            "w1": w1r, "w3": w3r, "w2": w2r,
            "s1": s1rr, "s3": s3rr, "s2": s2rr,
        })
    return in_maps


def _run(in_maps, trace=False):
    from concourse.bass_utils import run_bass_kernel_spmd
    C = _CACHE["C"]
    if _CACHE.get("nc_C") != C:
        _CACHE["nc"] = _build_nc(C=C)
        _CACHE["nc_C"] = C
    nc = _CACHE["nc"]
    return run_bass_kernel_spmd(nc, in_maps, list(range(E)), trace=trace)


def kernel(x, gate_w, w1, w3, w2, sw1, sw3, sw2):
    in_maps = _prep_inputs(x, gate_w, w1, w3, w2, sw1, sw3, sw2)
    res = _run(in_maps)
    sels, cnts = _CACHE["route"]
    total = np.zeros((T, D), np.float32)
    for r in res.results:
        total += r["os"].astype(np.float32)
    for e, r in enumerate(res.results):
        total[sels[e]] += r["oe"][:cnts[e]].astype(np.float32)
    return total.reshape(x.shape).astype(x.dtype)


# revision 19
# speedup vs baseline: 1.1182x; 1.0139x over previous
"""MoE FFN (SwiGLU, E=8 experts, top-2) + shared expert, expert-parallel
across 8 NeuronCores, with host-side token routing.

Strategy: the gate (softmax -> top-2 -> renormalize) is computed on host
with the exact same jax ops as the reference (on CPU), which yields per
expert the list of selected tokens and their combine weights.  Core e
owns expert e and computes the SwiGLU FFN only over the ~T*2/8 tokens
routed to it (gathered+padded to capacity C, a multiple of 128), scaled
by the combine weight on device.  Each core additionally computes a
344-row shard of the shared expert's hidden dim over all T tokens.
Host gathers: out = sum_cores shared_partial; out[idx_e] += expert_out_e.

All FFN matmuls run in bf16 with fp32 PSUM accumulation.  Expert hidden
is tiled 21x128+64 (no padding compute), shared shard 128+128+88.
Weight DMAs ride the Pool+ACT queues, x tiles the SP queue, output
tiles the ACT queue, so they overlap.  PSUM accumulation chains are
interleaved (rolling groups of 3 for h1/h3, offset dn-pairs for w2) so
consecutive matmuls never accumulate into the same PSUM bank - the
same-bank read-modify-write stall costs ~213ns per matmul.
"""
import numpy as np
from contextlib import ExitStack

D, E, T = 1024, 8, 4096
NK = 8                              # D / 128 contraction tiles
TN = 512                            # token tile (moving free axis)
NTN = T // TN                       # 8 shared-expert token tiles
H = 2752
HM_W = [128] * 21 + [64]            # expert hidden tile widths (sum 2752)
NHM = len(HM_W)
HS = 344                            # shared-expert hidden shard per core
HS_W = [128, 128, 88]               # shard tile widths (sum 344)
NHS = len(HS_W)
HSP = 384                           # padded shard rows in DRAM layout

_CACHE = {}


def _route(xt, gate_w):
    """Top-2 routing, mirroring reference ops bit-for-bit on CPU jax."""
    try:
        import jax
        import jax.numpy as jnp
        cpu = jax.devices("cpu")[0]
        with jax.default_device(cpu):
            logits = jnp.asarray(xt) @ jnp.asarray(gate_w).T
            scores = jax.nn.softmax(logits, axis=-1)
            tw, ti = jax.lax.top_k(scores, 2)
            tw = tw / (jnp.sum(tw, axis=-1, keepdims=True) + 1e-20)
        return np.asarray(tw), np.asarray(ti)
    except Exception:
        lg = xt.astype(np.float64) @ gate_w.astype(np.float64).T
        sc = np.exp(lg - lg.max(-1, keepdims=True))
        sc /= sc.sum(-1, keepdims=True)
        ti = np.argsort(-sc, axis=-1, kind="stable")[:, :2]
        tw = np.take_along_axis(sc, ti, axis=-1)
        tw = tw / (tw.sum(-1, keepdims=True) + 1e-20)
        return tw.astype(np.float32), ti.astype(np.int32)


def _build_nc(reps=1, C=None):
    import concourse.tile as tile
    from concourse import bacc, mybir

    if C is None:
        C = _CACHE.get("C", 1152)
    NET = (C + TN - 1) // TN            # expert token tiles (DRAM padded to 512)
    CP = NET * TN
    ET_W = [min(TN, C - t * TN) for t in range(NET)]   # compute widths

    f32 = mybir.dt.float32
    bf16 = mybir.dt.bfloat16
    ALU = mybir.AluOpType
    ACT = mybir.ActivationFunctionType

    nc = bacc.Bacc("TRN2", target_bir_lowering=False, debug=False, num_devices=8)

    xsr = nc.dram_tensor("xs", [128, NTN, NK, TN], bf16, kind="ExternalInput").ap()
    xer = nc.dram_tensor("xe", [128, NET, NK, TN], bf16, kind="ExternalInput").ap()
    cwr = nc.dram_tensor("cw", [128, CP // 128], f32, kind="ExternalInput").ap()
    w1r = nc.dram_tensor("w1", [128, NHM, NK, 128], bf16, kind="ExternalInput").ap()
    w3r = nc.dram_tensor("w3", [128, NHM, NK, 128], bf16, kind="ExternalInput").ap()
    w2r = nc.dram_tensor("w2", [128, NHM, 1024], bf16, kind="ExternalInput").ap()
    s1r = nc.dram_tensor("s1", [128, NHS, NK, 128], bf16, kind="ExternalInput").ap()
    s3r = nc.dram_tensor("s3", [128, NHS, NK, 128], bf16, kind="ExternalInput").ap()
    s2r = nc.dram_tensor("s2", [128, NHS, 1024], bf16, kind="ExternalInput").ap()
    oer = nc.dram_tensor("oe", [CP, D], bf16, kind="ExternalOutput").ap()
    osr = nc.dram_tensor("os", [T, D], bf16, kind="ExternalOutput").ap()

    with tile.TileContext(nc) as tc, ExitStack() as ctx:
        wpool = ctx.enter_context(tc.tile_pool(name="w", bufs=1))
        w1sb = wpool.tile([128, NHM, NK, 128], bf16, tag="w1")
        w3sb = wpool.tile([128, NHM, NK, 128], bf16, tag="w3")
        w2sb = wpool.tile([128, NHM, 1024], bf16, tag="w2")
        s1sb = wpool.tile([128, NHS, NK, 128], bf16, tag="s1")
        s3sb = wpool.tile([128, NHS, NK, 128], bf16, tag="s3")
        s2sb = wpool.tile([128, NHS, 1024], bf16, tag="s2")
        cwsb = wpool.tile([128, CP // 128], f32, tag="cw")

        # shared-expert weights first (needed by the first work items);
        # big expert weights spread across three DGE queues (Pool, DVE,
        # ACT) so they land in ~1/3 the time for single-shot latency
        nc.gpsimd.dma_start(s1sb[:], s1r[:])
        nc.gpsimd.dma_start(s3sb[:], s3r[:])
        nc.scalar.dma_start(s2sb[:], s2r[:])
        nc.gpsimd.dma_start(w1sb[:], w1r[:])
        nc.scalar.dma_start(w3sb[:], w3r[:])
        nc.gpsimd.dma_start(w2sb[:], w2r[:])
        nc.sync.dma_start(cwsb[:], cwr[:])

        xp = ctx.enter_context(tc.tile_pool(name="x", bufs=2))
        hp = ctx.enter_context(tc.tile_pool(name="h", bufs=1))
        shp = ctx.enter_context(tc.tile_pool(name="sh", bufs=2))
        silp = ctx.enter_context(tc.tile_pool(name="sil", bufs=2))
        obp = ctx.enter_context(tc.tile_pool(name="ob", bufs=3))
        pfw = ctx.enter_context(tc.tile_pool(name="pfw", bufs=1, space="PSUM"))
        pyo = ctx.enter_context(tc.tile_pool(name="pyo", bufs=2, space="PSUM"))
        # pfw: tags g0..g3 x1 buf = 4 banks; pyo: tags y0,y1 x2 bufs = 4 banks

        def emit_item(shared, idx):
            if shared:
                hw_list, wa, wb, wc = HS_W, s1sb, s3sb, s2sb
                xsrc, w = xsr[:, idx], TN
            else:
                hw_list, wa, wb, wc = HM_W, w1sb, w3sb, w2sb
                xsrc, w = xer[:, idx], ET_W[idx]
            nh = len(hw_list)

            xt = xp.tile([128, NK, TN], bf16, tag="x")
            nc.sync.dma_start(xt[:], xsrc)

            if shared:
                hbuf = shp.tile([128, NHS, TN], bf16, tag="shT")
            else:
                hbuf = hp.tile([128, NHM, TN], bf16, tag="hT")

            # h1/h3 PSUM chains run as rolling groups of 3 interleaved
            # chains: consecutive matmuls hit different PSUM banks with
            # different stationaries, dodging the same-bank accumulation
            # stall (427ns/MM sequential -> ~239ns/MM at 3-way).
            chains = []
            for hm in range(nh):
                chains.append((hm, 0))
                chains.append((hm, 1))
            sil_t = {}
            for c0 in range(0, len(chains), 3):
                grp = chains[c0:c0 + 3]
                tiles = []
                for s in range(len(grp)):
                    gt = f"g{(c0 + s) % 4}"   # 4 rotating bank slots
                    pt = pfw.tile([128, TN], f32, tag=gt, name=f"pt{s}")
                    tiles.append(pt)
                for kk in range(NK):
                    for s, (hm, kind) in enumerate(grp):
                        wh = hw_list[hm]
                        wsrc = wa if kind == 0 else wb
                        nc.tensor.matmul(tiles[s][:wh, :w],
                                         wsrc[:, hm, kk, :wh], xt[:, kk, :w],
                                         start=(kk == 0), stop=(kk == NK - 1))
                for s, (hm, kind) in enumerate(grp):
                    wh = hw_list[hm]
                    if kind == 0:
                        sil = silp.tile([128, TN], f32, tag="sil", name="sil")
                        nc.scalar.activation(sil[:wh, :w], tiles[s][:wh, :w],
                                             ACT.Silu)
                        sil_t[hm] = sil
                    else:
                        nc.vector.tensor_mul(hbuf[:wh, hm, :w],
                                             sil_t.pop(hm)[:wh, :w],
                                             tiles[s][:wh, :w])

            # w2: per tm-block, dn=0 / dn=1 chains interleaved with a
            # one-step offset so consecutive matmuls differ in both bank
            # and stationary.  Each tm's y1 epilogue is deferred into the
            # next tm's stream to avoid back-to-back same-bank matmuls.
            def stat(hk, tm):
                return hbuf[:hw_list[hk], hk, tm * 128:(tm + 1) * 128]

            def emit_consumers(tm, y0, y1, ob):
                for dn, yo in ((0, y0), (1, y1)):
                    osl = ob[:, dn * 512:(dn + 1) * 512]
                    if shared:
                        # DVE, not ACT: ACT already carries the silus plus
                        # output-DMA issue and would lag the PE here
                        nc.vector.tensor_copy(osl, yo[:])
                    else:
                        cb = idx * (TN // 128) + tm
                        nc.vector.tensor_scalar(osl, yo[:], cwsb[:, cb:cb + 1],
                                                None, op0=ALU.mult)
                t0 = idx * TN + tm * 128
                dst = osr[t0:t0 + 128, :] if shared else oer[t0:t0 + 128, :]
                nc.scalar.dma_start(dst, ob[:])

            pending = None   # (tm, y0, y1, ob) awaiting y1 epilogue
            for tm in range(w // 128):
                ob = obp.tile([128, D], bf16, tag="ob")
                y0 = pyo.tile([128, 512], f32, tag="y0")
                y1 = pyo.tile([128, 512], f32, tag="y1")
                nc.tensor.matmul(y0[:], stat(0, tm), wc[:hw_list[0], 0, 0:512],
                                 start=True, stop=(nh == 1))
                if pending is not None:
                    ptm, py0, py1, pob = pending
                    nc.tensor.matmul(py1[:], stat(nh - 1, ptm),
                                     wc[:hw_list[nh - 1], nh - 1, 512:1024],
                                     start=(nh == 1), stop=True)
                    emit_consumers(ptm, py0, py1, pob)
                for hk in range(1, nh):
                    nc.tensor.matmul(y0[:], stat(hk, tm),
                                     wc[:hw_list[hk], hk, 0:512],
                                     start=False, stop=(hk == nh - 1))
                    nc.tensor.matmul(y1[:], stat(hk - 1, tm),
                                     wc[:hw_list[hk - 1], hk - 1, 512:1024],
                                     start=(hk == 1), stop=False)
                pending = (tm, y0, y1, ob)
            ptm, py0, py1, pob = pending
            nc.tensor.matmul(py1[:], stat(nh - 1, ptm),
                             wc[:hw_list[nh - 1], nh - 1, 512:1024],
                             start=(nh == 1), stop=True)
            emit_consumers(ptm, py0, py1, pob)

        # interleave: 2 shared items first (their weights+x land first),
        # then alternate expert/shared so dependency bubbles overlap
        order = [("s", 0), ("s", 1)]
        si, ei = 2, 0
        while ei < NET or si < NTN:
            if ei < NET:
                order.append(("e", ei)); ei += 1
            if si < NTN:
                order.append(("s", si)); si += 1

        for _rep in range(reps):
            for kind, idx in order:
                emit_item(kind == "s", idx)

    nc.compile()
    return nc


def _prep_inputs(x, gate_w, w1, w3, w2, sw1, sw3, sw2):
    import ml_dtypes
    bf16 = ml_dtypes.bfloat16

    xf = np.ascontiguousarray(x.reshape(T, D).astype(np.float32))
    topk_w, topk_idx = _route(xf, np.asarray(gate_w, np.float32))

    sels, wsels = [], []
    for e in range(E):
        m = topk_idx == e                    # [T, 2]
        sel = np.nonzero(m.any(1))[0]
        wv = np.where(m[:, 0], topk_w[:, 0], topk_w[:, 1])[sel]
        sels.append(sel)
        wsels.append(wv.astype(np.float32))
    cnts = [len(s) for s in sels]
    C = max(512, -(-max(cnts) // 128) * 128)
    NET = (C + TN - 1) // TN
    CP = NET * TN

    _CACHE["C"] = C
    _CACHE["route"] = (sels, cnts)

    # [128p, NTN, NK, TN]: X[p,tn,kk,c] = xf[tn*TN+c, kk*128+p]
    xsr = np.ascontiguousarray(
        xf.reshape(NTN, TN, NK, 128).transpose(3, 0, 2, 1).astype(bf16))

    def pad_rows(a, n):
        return np.concatenate(
            [a, np.zeros((n - a.shape[0],) + a.shape[1:], a.dtype)], 0)

    in_maps = []
    for e in range(E):
        xg = pad_rows(xf[sels[e]], CP)                    # [CP, D]
        xer = np.ascontiguousarray(
            xg.reshape(NET, TN, NK, 128).transpose(3, 0, 2, 1).astype(bf16))
        cwv = pad_rows(wsels[e], CP)                      # [CP]
        cwr = np.ascontiguousarray(cwv.reshape(CP // 128, 128).T)

        w1p = pad_rows(np.asarray(w1[e], np.float32), NHM * 128)   # [2816, D]
        w3p = pad_rows(np.asarray(w3[e], np.float32), NHM * 128)
        w2p = np.asarray(w2[e], np.float32)                        # [D, H]
        w2p = np.concatenate(
            [w2p, np.zeros((D, NHM * 128 - H), np.float32)], 1)

        w1r = np.ascontiguousarray(
            w1p.T.reshape(NK, 128, NHM, 128).transpose(1, 2, 0, 3).astype(bf16))
        w3r = np.ascontiguousarray(
            w3p.T.reshape(NK, 128, NHM, 128).transpose(1, 2, 0, 3).astype(bf16))
        w2r = np.ascontiguousarray(
            w2p.T.reshape(NHM, 128, D).transpose(1, 0, 2).astype(bf16))

        s1p = pad_rows(np.asarray(sw1[e * HS:(e + 1) * HS], np.float32), HSP)
        s3p = pad_rows(np.asarray(sw3[e * HS:(e + 1) * HS], np.float32), HSP)
        s2p = np.asarray(sw2[:, e * HS:(e + 1) * HS], np.float32)
        s2p = np.concatenate([s2p, np.zeros((D, HSP - HS), np.float32)], 1)

        s1rr = np.ascontiguousarray(
            s1p.T.reshape(NK, 128, NHS, 128).transpose(1, 2, 0, 3).astype(bf16))
        s3rr = np.ascontiguousarray(
            s3p.T.reshape(NK, 128, NHS, 128).transpose(1, 2, 0, 3).astype(bf16))
        s2rr = np.ascontiguousarray(
            s2p.T.reshape(NHS, 128, D).transpose(1, 0, 2).astype(bf16))

        in_maps.append({
            "xs": xsr, "xe": xer, "cw": cwr,
            "w1": w1r, "w3": w3r, "w2": w2r,
            "s1": s1rr, "s3": s3rr, "s2": s2rr,
        })
    return in_maps


def _run(in_maps, trace=False):
    from concourse.bass_utils import run_bass_kernel_spmd
    C = _CACHE["C"]
    if _CACHE.get("nc_C") != C:
        _CACHE["nc"] = _build_nc(C=C)
        _CACHE["nc_C"] = C
    nc = _CACHE["nc"]
    return run_bass_kernel_spmd(nc, in_maps, list(range(E)), trace=trace)


def kernel(x, gate_w, w1, w3, w2, sw1, sw3, sw2):
    in_maps = _prep_inputs(x, gate_w, w1, w3, w2, sw1, sw3, sw2)
    res = _run(in_maps)
    sels, cnts = _CACHE["route"]
    total = np.zeros((T, D), np.float32)
    for r in res.results:
        total += r["os"].astype(np.float32)
    for e, r in enumerate(res.results):
        total[sels[e]] += r["oe"][:cnts[e]].astype(np.float32)
    return total.reshape(x.shape).astype(x.dtype)
